# revision 10
# baseline (speedup 1.0000x reference)
"""Trainium2 Bass kernel for CausalSelfAttention with KV cache (B=16, S=16,
D=1024, H=16, hd=64, PAST=4096), non-causal attention, fp32.

Sharding: tensor-parallel across heads — 2 heads per core on 8 cores.
Each core: projects q/k/v for its 2 heads, attends over the 4096-entry
cache + 16 new keys, and computes a partial output projection.
Host: preps transposed layouts, sums the 8 partial y outputs, and
assembles the concatenated k/v outputs.

Math layout per core (all fp32):
  qT/kT/vT  [128 (2h*64d), 256 (16b*16s)]  = W_local @ x^T + b
  scores^T  [l, s] per (b,h) = (K^T tile).T @ qT slice       (PE)
  exp       scalar engine, scale=1/8, no max-subtraction (scores are O(4))
  o_unnorm  [16 s, 65] = expT.T @ [V | ones]  (col 64 = softmax denom)
  o         = o_unnorm * recip(denom)   -> PE transpose -> oT [128, 256]
  y_partial [256, 1024] = oT.T @ Wo_local^T   (host adds bias + reduces)
"""

import numpy as np

B, S, D = 16, 16, 1024
H, HD, PAST = 16, 64, 4096
NCORES = 8
L = PAST + S  # 4112

_CACHE = {}


def _build_program():
    import concourse.bacc as bacc
    import concourse.mybir as mybir
    import concourse.tile as tile
    from concourse import masks

    F32 = mybir.dt.float32
    Exp = mybir.ActivationFunctionType.Exp

    # Bacc (not raw Bass): its compile() runs the wait-legalization passes
    # (move_matmul_waits_to_ldweights / generate_event_semaphores) without
    # which multi-wait PE instructions fail walrus codegen.
    nc = bacc.Bacc(
        "TRN2",
        target_bir_lowering=False,
        debug=False,
        enable_asserts=False,
        num_devices=NCORES,
    )

    xt_d = nc.dram_tensor("xt", [128, 8, 256], F32, kind="ExternalInput")
    wq_d = nc.dram_tensor("wq", [128, 8, 128], F32, kind="ExternalInput")
    wk_d = nc.dram_tensor("wk", [128, 8, 128], F32, kind="ExternalInput")
    wv_d = nc.dram_tensor("wv", [128, 8, 128], F32, kind="ExternalInput")
    wo_d = nc.dram_tensor("wo", [64, 2, 1024], F32, kind="ExternalInput")
    bq_d = nc.dram_tensor("bq", [128, 1], F32, kind="ExternalInput")
    bk_d = nc.dram_tensor("bk", [128, 1], F32, kind="ExternalInput")
    bv_d = nc.dram_tensor("bv", [128, 1], F32, kind="ExternalInput")
    kt_d = nc.dram_tensor("kt", [16, 128, 4096], F32, kind="ExternalInput")
    vx_d = nc.dram_tensor("vx", [16, 128, 2, 32, 65], F32, kind="ExternalInput")

    y_d = nc.dram_tensor("y", [2, 128, 1024], F32, kind="ExternalOutput")
    kn_d = nc.dram_tensor("knew", [128, 256], F32, kind="ExternalOutput")
    vn_d = nc.dram_tensor("vnew", [128, 256], F32, kind="ExternalOutput")

    with tile.TileContext(nc) as tc:
        with (
            tc.tile_pool(name="persist", bufs=1) as P,
            tc.tile_pool(name="pp", bufs=1, space="PSUM") as PP,
        ):
            ident = P.tile([128, 128], F32, tag="ident")
            masks.make_identity(nc, ident[:])
            qt_sb = P.tile([128, 256], F32, tag="qt")
            kt_sb = P.tile([128, 256], F32, tag="kts")
            vt_sb = P.tile([128, 256], F32, tag="vts")
            vnew_sb = P.tile([16, 16, 130], F32, tag="vnew")
            # per-head o^T accumulators (matmul outputs must start at PSUM
            # partition 0, so the two heads get separate banks)
            oT_ps = [PP.tile([64, 256], F32, tag=f"oT{h}", name=f"oT{h}") for h in range(2)]

            # ---------------- phase 0: qkv projections ----------------
            with (
                tc.tile_pool(name="p0", bufs=1) as p0,
                tc.tile_pool(name="p0p", bufs=2, space="PSUM") as p0p,
                tc.tile_pool(name="ptv", bufs=2, space="PSUM") as ptv,
            ):
                xt = p0.tile([128, 8, 256], F32, tag="xt")
                nc.sync.dma_start(xt[:], xt_d[:])
                wts, bss = {}, {}
                for nm, wd, bd in (
                    ("q", wq_d, bq_d),
                    ("k", wk_d, bk_d),
                    ("v", wv_d, bv_d),
                ):
                    w = p0.tile([128, 8, 128], F32, tag="w" + nm)
                    nc.sync.dma_start(w[:], wd[:])
                    bt = p0.tile([128, 1], F32, tag="b" + nm)
                    nc.sync.dma_start(bt[:], bd[:])
                    wts[nm], bss[nm] = w, bt

                for nm, dst in (("q", qt_sb), ("k", kt_sb), ("v", vt_sb)):
                    ps = p0p.tile([128, 256], F32, tag="proj")
                    for t in range(8):
                        nc.tensor.matmul(
                            ps[:],
                            wts[nm][:, t, :],
                            xt[:, t, :],
                            start=(t == 0),
                            stop=(t == 7),
                        )
                    nc.vector.tensor_scalar_add(dst[:], ps[:], bss[nm][:])

                # new-key k/v outputs (host re-assembles)
                nc.sync.dma_start(kn_d[:], kt_sb[:])
                nc.sync.dma_start(vn_d[:], vt_sb[:])

                # per-batch natural-layout v_new (for the 33rd attention chunk)
                nc.vector.memset(vnew_sb[:, :, 64:65], 1.0)
                nc.vector.memset(vnew_sb[:, :, 129:130], 1.0)
                for g in range(4):
                    pv = ptv.tile([16, 512], F32, tag="tv")
                    for i in range(4):
                        b = 4 * g + i
                        nc.tensor.transpose(
                            pv[0:16, 128 * i : 128 * i + 128],
                            vt_sb[:, 16 * b : 16 * b + 16],
                            ident[:],
                        )
                    for i in range(4):
                        b = 4 * g + i
                        nc.scalar.copy(
                            vnew_sb[:, b, 0:64], pv[0:16, 128 * i : 128 * i + 64]
                        )
                        nc.scalar.copy(
                            vnew_sb[:, b, 65:129],
                            pv[0:16, 128 * i + 64 : 128 * i + 128],
                        )

            # ---------------- phase 1: attention over cache ----------------
            with (
                tc.tile_pool(name="kv", bufs=2) as kvp,
                tc.tile_pool(name="att", bufs=3) as att,
                tc.tile_pool(name="ps_s", bufs=3, space="PSUM") as ps_s,
                tc.tile_pool(name="ps_o", bufs=2, space="PSUM") as ps_o,
            ):
                for b in range(16):
                    KT = kvp.tile([128, 4112], F32, tag="KT")
                    nc.sync.dma_start(KT[:, 0:4096], kt_d[b])
                    nc.scalar.copy(KT[:, 4096:4112], kt_sb[:, 16 * b : 16 * b + 16])
                    V = kvp.tile([128, 2, 32, 65], F32, tag="V")
                    nc.sync.dma_start(V[:], vx_d[b])
                    for h in range(2):
                        hp = slice(64 * h, 64 * h + 64)
                        qs = qt_sb[hp, 16 * b : 16 * b + 16]
                        expT = att.tile([128, 528], F32, tag="expT")
                        for bank in range(4):
                            ps = ps_s.tile([128, 144], F32, tag="s")
                            for j in range(8):
                                ch = 8 * bank + j
                                nc.tensor.matmul(
                                    ps[:, 16 * j : 16 * j + 16],
                                    KT[hp, 128 * ch : 128 * ch + 128],
                                    qs,
                                    start=True,
                                    stop=True,
                                )
                            if bank == 3:
                                nc.tensor.matmul(
                                    ps[0:16, 128:144],
                                    KT[hp, 4096:4112],
                                    qs,
                                    start=True,
                                    stop=True,
                                )
                            nc.scalar.activation(
                                expT[:, 128 * bank : 128 * bank + 128],
                                ps[:, 0:128],
                                Exp,
                                scale=0.125,
                            )
                            if bank == 3:
                                nc.scalar.activation(
                                    expT[0:16, 512:528],
                                    ps[0:16, 128:144],
                                    Exp,
                                    scale=0.125,
                                )
                        po = ps_o.tile([16, 65], F32, tag="o")
                        for ch in range(32):
                            nc.tensor.matmul(
                                po[:],
                                expT[:, 16 * ch : 16 * ch + 16],
                                V[:, h, ch, :],
                                start=(ch == 0),
                                stop=False,
                            )
                        nc.tensor.matmul(
                            po[:],
                            expT[0:16, 512:528],
                            vnew_sb[:, b, 65 * h : 65 * h + 65],
                            start=False,
                            stop=True,
                        )
                        rcp = att.tile([16, 1], F32, tag="rcp")
                        nc.vector.reciprocal(rcp[:], po[0:16, 64:65])
                        onat = att.tile([16, 64], F32, tag="onat")
                        nc.vector.tensor_scalar_mul(onat[:], po[0:16, 0:64], rcp[:])
                        nc.tensor.transpose(
                            oT_ps[h][:, 16 * b : 16 * b + 16],
                            onat[:],
                            ident[0:16, 0:16],
                        )

            # ---------------- phase 2: output projection ----------------
            with (
                tc.tile_pool(name="p2", bufs=2) as p2,
                tc.tile_pool(name="p2p", bufs=2, space="PSUM") as p2p,
            ):
                oT_sb = [p2.tile([64, 256], F32, tag=f"oTsb{h}", name=f"oTsb{h}") for h in range(2)]
                for h in range(2):
                    nc.scalar.copy(oT_sb[h][:], oT_ps[h][:])
                wo_sb = p2.tile([64, 2, 1024], F32, tag="wo")
                nc.sync.dma_start(wo_sb[:], wo_d[:])
                for sc in range(2):
                    ysb = p2.tile([128, 1024], F32, tag="y")
                    for nh in range(2):
                        py = p2p.tile([128, 512], F32, tag="py")
                        for h in range(2):
                            nc.tensor.matmul(
                                py[:],
                                oT_sb[h][:, 128 * sc : 128 * sc + 128],
                                wo_sb[:, h, 512 * nh : 512 * nh + 512],
                                start=(h == 0),
                                stop=(h == 1),
                            )
                        nc.scalar.copy(ysb[:, 512 * nh : 512 * nh + 512], py[:])
                    nc.sync.dma_start(y_d[sc], ysb[:])

    nc.compile()
    return nc


def _prep_inputs(x, cache_k, cache_v, qkv_w, qkv_b, out_w):
    """Build the 8 per-core input maps (host-side sharding + re-layout)."""
    f32 = np.float32
    x_flat = np.asarray(x, f32).reshape(B * S, D)
    xt_host = np.ascontiguousarray(
        x_flat.T.reshape(8, 128, B * S).transpose(1, 0, 2)
    )
    qkv_w = np.asarray(qkv_w, f32)
    qkv_b = np.asarray(qkv_b, f32)
    out_w = np.asarray(out_w, f32)
    cache_k = np.asarray(cache_k, f32)
    cache_v = np.asarray(cache_v, f32)

    in_maps = []
    for c in range(NCORES):
        r0, r1 = 2 * c * HD, (2 * c + 2) * HD
        m = {"xt": xt_host}
        for nm, off in (("wq", 0), ("wk", D), ("wv", 2 * D)):
            wl = qkv_w[off + r0 : off + r1]  # [128, 1024]
            m[nm] = np.ascontiguousarray(
                wl.T.reshape(8, 128, 128).transpose(1, 0, 2)
            )
        # [64 d, 2 h, 1024 dout]
        m["wo"] = np.ascontiguousarray(
            out_w[:, r0:r1].T.reshape(2, HD, D).transpose(1, 0, 2)
        )
        m["bq"] = np.ascontiguousarray(qkv_b[r0:r1][:, None])
        m["bk"] = np.ascontiguousarray(qkv_b[D + r0 : D + r1][:, None])
        m["bv"] = np.ascontiguousarray(qkv_b[2 * D + r0 : 2 * D + r1][:, None])
        ck = cache_k[:, 2 * c : 2 * c + 2]  # [16, 2, 4096, 64]
        m["kt"] = np.ascontiguousarray(ck.transpose(0, 1, 3, 2)).reshape(
            16, 128, 4096
        )
        cv = cache_v[:, 2 * c : 2 * c + 2].reshape(16, 2, 32, 128, 64)
        vx = np.ones((16, 128, 2, 32, 65), f32)
        vx[..., :64] = cv.transpose(0, 3, 1, 2, 4)
        m["vx"] = vx
        in_maps.append(m)
    return in_maps


def _run(in_maps, trace=False):
    from concourse.bass_utils import run_bass_kernel_spmd

    if "nc" not in _CACHE:
        _CACHE["nc"] = _build_program()
    return run_bass_kernel_spmd(
        _CACHE["nc"], in_maps, core_ids=list(range(NCORES)), trace=trace
    )


def kernel(x, cache_k, cache_v, qkv_w, qkv_b, out_w, out_b, _trace=False):
    in_maps = _prep_inputs(x, cache_k, cache_v, qkv_w, qkv_b, out_w)
    res = _run(in_maps, trace=_trace)
    results = res.results
    if _trace:
        kernel.last_exec_time_ns = res.exec_time_ns
        kernel.last_result = res

    f32 = np.float32
    y = np.zeros((B * S, D), f32)
    for c in range(NCORES):
        y += results[c]["y"].reshape(B * S, D)
    y = (y + np.asarray(out_b, f32)).reshape(B, S, D)

    k_full = np.empty((B, H, L, HD), f32)
    v_full = np.empty((B, H, L, HD), f32)
    k_full[:, :, :PAST] = cache_k
    v_full[:, :, :PAST] = cache_v
    for c in range(NCORES):
        # [128 (2h*64d), 256 (16b*16s)] -> [b, h, s, d]
        kn = results[c]["knew"].reshape(2, HD, B, S).transpose(2, 0, 3, 1)
        vn = results[c]["vnew"].reshape(2, HD, B, S).transpose(2, 0, 3, 1)
        k_full[:, 2 * c : 2 * c + 2, PAST:] = kn
        v_full[:, 2 * c : 2 * c + 2, PAST:] = vn
    return y, k_full, v_full
